# revision 28
# baseline (speedup 1.0000x reference)
"""Trainium2 Bass kernel for 4-head spatial attention score softmax.

Reference computation:
    qk = einsum('bcxy,oc->boxy', fmap[1,256,64,64], W_qk[1024,256])
    q, k = split(qk, 2, axis=1)             # each [1, 512, 64, 64]
    q = q reshaped to heads, scaled by 128^-0.5
    sim[b,h,xy,uv] = q . k  (contraction over dim_head=128)
    out = softmax(sim, axis=-1)             # [1, 4, 4096, 4096] f32
    (b=1, 4096 = 64*64 spatial positions)

Sharding: 8 cores = 4 heads x 2 query-halves. Each core projects q for its
2048 query columns + k for all 4096 columns (PE matmuls over the channel
dim), computes scores with fp16 matmuls, softmax (exp on ScalarE with
accumulated row sums, normalize on VectorE), and streams its [2048, 4096]
f32 output slab to HBM.

The run is DMA-bound: the 33.55MB output write streams at the ~350-424GB/s
per-core cap (all 16 DMA engines 99% busy) with zero gaps, so total time =
first-store-time + stream. The prologue (first store at ~28us vs 38us for
the f32r baseline) is engineered around trace-observed constraints:
  - All input loads ride ONE ring (Sync - it arms earliest, ~8.9us;
    multiple rings fight for the same 16 DMA engines in unpredictable
    order) in exact consumption order: w, then 4 fmap chunks. The ring
    drains ~8us before the first store is enqueued on it. fmap is packed
    on host so every chunk is one 4KB descriptor per partition, and
    pre-rotated per core so this core's query columns are always
    [0, 2048) (static offsets; the host un-rotates the output columns
    during assembly, a half-swap).
  - PE executes in order at ~215ns per 512-wide fp16 matmul when hot;
    the HAM governor halves the clock ~3.4us after any PE idle window,
    so the warmup burst is sized to end right as chunk 0 lands and the
    projections are emitted in chunk-arrival order with no filler.
  - Only q[0:384] (query tiles 0-2) is projected+copied before tile 0;
    the remaining 1792 q columns are projected during the exp/stream
    window. This keeps the pre-tile-0 spine to: last chunk -> kproj3 ->
    k3 cast (VectorE) -> scores h1 -> exp tail (ScalarE) -> sum ->
    normalize -> store. Tile 0's exp is split 2048|1024|1024 so only a
    1024-wide exp depends on the last k chunk's scores.
  - With 2 PSUM slots of [128,2048] (8 banks), each tile's matmuls wait
    on the previous-but-one tile's consumer; emission order is chosen so
    those waits coincide with real data dependencies.
  - tile 0 normalizes + stores in 1024-col quarters (first bytes to HBM
    ~0.9us after the row sum), tiles 1-3 in halves; steady-state tiles
    do one full-row normalize + store, which the stream cadence hides.
"""

import numpy as np

import concourse.bacc as bacc
import concourse.mybir as mybir
import concourse.tile as tile
from concourse import bass_utils

HEADS = 4
DIM_HEAD = 128
C = 256          # input channels
XY = 4096        # 64*64 spatial positions
QCHUNK = 2048    # query positions per core
N_CORES = 8
SCALE = DIM_HEAD ** -0.5

F32 = mybir.dt.float32

# dtype of everything the PE touches (fmap, weights, q, k). 16-bit halves
# both the HBM load bytes and the PE streaming cost vs f32r, and enables
# fast weight load. fp16 over bf16: all values are O(1), so the e5m10
# mantissa (exact inside the PE's FP22) cuts quantization error ~8x.
# NOTE: both matmul operands MUST share one dtype - mixing fp16/bf16 in a
# single matmul hard-crashes the device (NRT_EXEC_UNIT_UNRECOVERABLE).
QK_DT = mybir.dt.float16

# load chunks: 4x1024 columns; q is chunks 0-1
KCH = 1024
CHUNKS = [1024, 1024, 1024, 1024]
CHUNK_OFF = [0, 1024, 2048, 3072]


def _emit(tc, fmap_p, wqkt, out):
    nc = tc.nc

    with tc.tile_pool(name="consts", bufs=1) as consts:
        w_sb = consts.tile([128, 2, 2 * DIM_HEAD], QK_DT)
        # fmap column chunk g: [128p, a, col] at fk_sb[:, 2*off : 2*(off+w)]
        fk_sb = consts.tile([128, 2 * XY], QK_DT)
        warm_sb = consts.tile([128, 512], QK_DT)
        q_sb = consts.tile([128, QCHUNK], QK_DT)  # [d, x] for this core's queries
        k_sb = consts.tile([128, XY], QK_DT)      # [d, uv]

        # memset first so the PE warmup isn't stuck behind dma issues
        nc.vector.memset(warm_sb, 0.0)

        # All input loads ride ONE ring - the Sync ring, which arms
        # earliest (~8.9us; Scalar/GpSimd rings arm 2-4us later and
        # multiple rings fight for the same 16 DMA engines in
        # unpredictable order, which repeatedly starved the first
        # chunk). A single ring delivers w + chunks in exact
        # consumption order at the ~260GB/s read cap, and drains ~6us
        # before the first store is enqueued on it.
        nc.sync.dma_start(out=w_sb,
                          in_=wqkt.rearrange("p (a d) -> p a d", a=2))
        # two 8KB-descriptor loads (cols 0:2048 | 2048:4096): bigger
        # descriptors lift the HBM read rate toward the write-class
        # 400GB/s; the packing already makes each half contiguous per
        # partition, and the projections still consume in 1024 chunks.
        for lo in (0, 2 * XY // 2):
            nc.sync.dma_start(out=fk_sb[:, lo:lo + XY],
                              in_=fmap_p[:, lo:lo + XY])

        # One PSUM pool + tag for warmup, projections, and scores: the
        # 2-slot rotation makes each new tile wait only on the
        # previous-but-one tile's consumer.
        with tc.tile_pool(name="ps", bufs=2, space="PSUM") as ps_pool, \
             tc.tile_pool(name="soft", bufs=6) as soft_pool, \
             tc.tile_pool(name="small", bufs=4) as small_pool:
            # PE warmup: dummy matmuls with no load deps ramp the HAM
            # clock gate to 2.4 GHz; sized to end near chunk 0's arrival
            # (PE is in-order, so oversizing delays the projections).
            warm_ps = ps_pool.tile([128, 2048], F32, tag="ps")
            for i in range(8):
                nc.tensor.matmul(warm_ps[:, 0:512], lhsT=warm_sb[:, 0:128],
                                 rhs=warm_sb, start=True, stop=True)

            # ---- per-chunk projections: out[d, n] = sum_c W^T[c, d] * fmap[c, n]
            # Copy-engine split (DVE: k0, k1, k3 / ACT: q0, q1, k2) keeps
            # the two copy streams parallel AND leaves ScalarE free from
            # ~17.6us on for the tile-0 exp chain.
            def emit_proj(g, which, copy_eng):
                off, width = CHUNK_OFF[g], CHUNKS[g]
                dlo = 0 if which == "q" else DIM_HEAD
                ps_p = ps_pool.tile([128, 2048], F32, tag="ps",
                                    name=f"ps_{which}{g}")
                for j in range(width // 512):
                    osl = slice(j * 512, (j + 1) * 512)
                    for a in range(2):
                        fsl = slice(2 * off + a * width + j * 512,
                                    2 * off + a * width + (j + 1) * 512)
                        nc.tensor.matmul(ps_p[:, osl],
                                         lhsT=w_sb[:, a, dlo:dlo + DIM_HEAD],
                                         rhs=fk_sb[:, fsl],
                                         start=(a == 0), stop=(a == 1))
                dst = q_sb if which == "q" else k_sb
                if copy_eng == "act":
                    nc.scalar.copy(dst[:, off:off + width], ps_p[:, 0:width])
                else:
                    nc.vector.tensor_copy(dst[:, off:off + width],
                                          ps_p[:, 0:width])

            def emit_score_half(qsl, half, name):
                ps = ps_pool.tile([128, 2048], F32, tag="ps", name=name)
                for j in range(4):
                    osl = slice(j * 512, (j + 1) * 512)
                    ksl = slice(half * 2048 + j * 512,
                                half * 2048 + (j + 1) * 512)
                    nc.tensor.matmul(ps[:, osl], lhsT=qsl,
                                     rhs=k_sb[:, ksl],
                                     start=True, stop=True)
                return ps

            # Projection of q is SPLIT: only columns [0, 256) (tiles 0-1)
            # are projected + copied before tile 0's chain; the remaining
            # 1792 columns are projected after tile 0 is emitted, during
            # the exp/stream window (they aren't consumed until query
            # tile 2, ~10us later). This takes two 1us-class copies off
            # the pre-tile-0 critical spine.
            def emit_qpart(lo, width, name, copy_eng):
                ps_p = ps_pool.tile([128, 2048], F32, tag="ps", name=name)
                # psum matmul outputs must stay inside one 512-col bank
                o = 0
                while o < width:
                    w2 = min(512, width - o)
                    for a in range(2):
                        # q column lo+o lives in load chunk g
                        g = (lo + o) // KCH
                        coff = lo + o - g * KCH
                        fsl = slice(2 * g * KCH + a * KCH + coff,
                                    2 * g * KCH + a * KCH + coff + w2)
                        nc.tensor.matmul(ps_p[:, o:o + w2],
                                         lhsT=w_sb[:, a, 0:DIM_HEAD],
                                         rhs=fk_sb[:, fsl],
                                         start=(a == 0), stop=(a == 1))
                    o += w2
                if copy_eng == "act":
                    nc.scalar.copy(q_sb[:, lo:lo + width], ps_p[:, 0:width])
                else:
                    nc.vector.tensor_copy(q_sb[:, lo:lo + width],
                                          ps_p[:, 0:width])

            # Dependency-ordered, with tile 0's first score half emitted
            # BEFORE the last k projection: the 2-slot PSUM rotation then
            # lets its matmuls run as soon as k[0:2048] is cast, and the
            # exp chain on ScalarE starts ~2us earlier. Tile 0's exp is
            # split [0:2048 | 2048:3072 | 3072:4096] so only a 1024-wide
            # exp remains after the last k chunk's scores.
            # chunks 0+1 arrive in one DMA, so their casts run in
            # parallel on VectorE + ScalarE instead of serializing
            emit_proj(0, "k", "dve")
            emit_proj(1, "k", "act")
            emit_qpart(0, 384, "ps_q0a", "act")
            emit_proj(2, "k", "dve")
            q0sl = q_sb[:, 0:128]
            et0 = soft_pool.tile([128, XY], F32, tag="et")
            pp0 = small_pool.tile([128, 4], F32, tag="pp")
            ps_h0 = emit_score_half(q0sl, 0, "ps_t0h0")
            nc.scalar.activation(out=et0[:, 0:2048], in_=ps_h0,
                                 func=mybir.ActivationFunctionType.Exp,
                                 accum_out=pp0[:, 0:1])
            emit_proj(3, "k", "dve")
            ps_h1 = emit_score_half(q0sl, 1, "ps_t0h1")
            nc.scalar.activation(out=et0[:, 2048:3072], in_=ps_h1[:, 0:1024],
                                 func=mybir.ActivationFunctionType.Exp,
                                 accum_out=pp0[:, 1:2])
            nc.scalar.activation(out=et0[:, 3072:4096], in_=ps_h1[:, 1024:2048],
                                 func=mybir.ActivationFunctionType.Exp,
                                 accum_out=pp0[:, 2:3])
            den0 = small_pool.tile([128, 1], F32, tag="den")
            nc.vector.tensor_reduce(den0, pp0[:, 0:3],
                                    axis=mybir.AxisListType.X,
                                    op=mybir.AluOpType.add)
            nc.vector.reciprocal(den0, den0)
            # normalize + store in pieces (512,512,1024,1024,1024): the
            # first bytes reach HBM ~0.6us after the row sum
            p_lo = 0
            for pw in (512, 512, 1024, 1024, 1024):
                sl2 = slice(p_lo, p_lo + pw)
                nc.vector.tensor_scalar_mul(et0[:, sl2], et0[:, sl2], den0)
                nc.sync.dma_start(out=out[0:128, sl2], in_=et0[:, sl2])
                p_lo += pw

            # ---- scores + softmax, query tiles 1-15 ----
            def emit_tile(qt):
                qsl = q_sb[:, qt * 128:(qt + 1) * 128]
                et = soft_pool.tile([128, XY], F32, tag="et")
                pp = small_pool.tile([128, 4], F32, tag="pp")
                for half in range(2):
                    ps = emit_score_half(qsl, half, f"ps_t{qt}h{half}")
                    # exp straight out of PSUM, with per-row partial sums
                    # accumulated for free.
                    nc.scalar.activation(
                        out=et[:, half * 2048:(half + 1) * 2048],
                        in_=ps,
                        func=mybir.ActivationFunctionType.Exp,
                        accum_out=pp[:, half:half + 1])
                den = small_pool.tile([128, 1], F32, tag="den")
                nc.vector.tensor_add(den, pp[:, 0:1], pp[:, 1:2])
                nc.vector.reciprocal(den, den)
                # tiles 1-3 store in halves (the stream is still ramping
                # and tile 2's full-row normalize would leave a ~1.4us
                # stream gap); later tiles do one full-row store.
                npieces = 2 if qt <= 3 else 1
                pw = XY // npieces
                for p2 in range(npieces):
                    sl2 = slice(p2 * pw, (p2 + 1) * pw)
                    nc.vector.tensor_scalar_mul(et[:, sl2], et[:, sl2], den)
                    nc.sync.dma_start(out=out[qt * 128:(qt + 1) * 128, sl2],
                                      in_=et[:, sl2])

            # deferred q projections slot into the exp/stream window:
            # tiles 1-2 are covered by q0a; tile 3 needs qr0 (~24us,
            # copied on ScalarE right after tile 0's exps), tile 8 needs
            # qr1 (~55us, copied on VectorE after tile 0's normalizes).
            emit_qpart(384, 640, "ps_qr0", "dve")
            emit_tile(1)
            emit_qpart(1024, 1024, "ps_qr1", "dve")
            for qt in range(2, QCHUNK // 128):
                emit_tile(qt)


def build_program():
    nc = bacc.Bacc("TRN2", target_bir_lowering=False, debug=False,
                   enable_asserts=False)
    fmap_p = nc.dram_tensor("fmap_p", [128, 2 * XY], QK_DT,
                            kind="ExternalInput").ap()
    wqkt = nc.dram_tensor("wqkt", [128, 4 * DIM_HEAD], QK_DT,
                          kind="ExternalInput").ap()
    out = nc.dram_tensor("out", [QCHUNK, XY], F32, kind="ExternalOutput").ap()

    with tile.TileContext(nc) as tc:
        _emit(tc, fmap_p, wqkt, out)
    nc.compile()
    return nc


_CACHE = {}


def _get_nc():
    if "nc" not in _CACHE:
        _CACHE["nc"] = build_program()
    return _CACHE["nc"]


def _pack_fmap(fm):
    """[256, 4096] fp16 -> [128, 8192] where partition p holds channels p
    (a=0) and 128+p (a=1), and each load chunk [off, off+width) is
    contiguous per partition as [a=0 cols | a=1 cols]."""
    t = fm.reshape(2, 128, XY)                        # [a, p, n]
    packed = np.empty((128, 2 * XY), dtype=np.float16)
    for off, width in zip(CHUNK_OFF, CHUNKS):
        seg = t[:, :, off:off + width]                # [a, p, w]
        packed[:, 2 * off:2 * (off + width)] = (
            seg.transpose(1, 0, 2).reshape(128, 2 * width))
    return packed


def make_in_maps(fmap, W_qk):
    fm = np.asarray(fmap, dtype=np.float32).reshape(C, XY)
    # Core 2h+1 sees fmap columns rotated left by QCHUNK, so its query
    # columns sit at [0, 2048) like core 2h's. 4096-col rotation by 2048
    # == swapping the two column halves.
    fm_rot = np.concatenate([fm[:, QCHUNK:], fm[:, :QCHUNK]],
                            axis=1).astype(np.float16)
    fm = fm.astype(np.float16)
    packed = _pack_fmap(fm)
    packed_rot = _pack_fmap(fm_rot)
    W = np.asarray(W_qk, dtype=np.float32)
    in_maps = []
    for core in range(N_CORES):
        hd, qhalf = divmod(core, 2)
        wq = W[hd * DIM_HEAD:(hd + 1) * DIM_HEAD] * np.float32(SCALE)
        wk = W[HEADS * DIM_HEAD + hd * DIM_HEAD:
               HEADS * DIM_HEAD + (hd + 1) * DIM_HEAD]
        # [c, dq|dk] -> [128, 2, 256]: partition p holds channels p and
        # 128+p as two contiguous 512B halves (one 1KB descriptor each)
        wcat = np.concatenate([wq.T, wk.T], axis=1).astype(np.float16)
        in_maps.append({
            "fmap_p": packed_rot if qhalf else packed,
            "wqkt": np.ascontiguousarray(
                wcat.reshape(2, 128, 256).transpose(1, 0, 2).reshape(128, 512)),
        })
    return in_maps


def assemble(per_core_outs):
    out = np.empty((HEADS, XY, XY), dtype=np.float32)
    for core in range(N_CORES):
        hd, qhalf = divmod(core, 2)
        slab = per_core_outs[core]
        rows = slice(qhalf * QCHUNK, (qhalf + 1) * QCHUNK)
        if qhalf:
            # un-rotate the uv columns (slab col j = true col (j+2048)%4096)
            out[hd, rows, :QCHUNK] = slab[:, QCHUNK:]
            out[hd, rows, QCHUNK:] = slab[:, :QCHUNK]
        else:
            out[hd, rows, :] = slab
    return out.reshape(1, HEADS, XY, XY)


def kernel(fmap, W_qk, trace=False):
    nc = _get_nc()
    in_maps = make_in_maps(fmap, W_qk)
    res = bass_utils.run_bass_kernel_spmd(
        nc, in_maps, core_ids=list(range(N_CORES)), trace=trace)
    out = assemble([res.results[c]["out"] for c in range(N_CORES)])
    if trace:
        kernel.last_exec_time_ns = res.exec_time_ns
        kernel.last_results = res
    return out
